# revision 6
# baseline (speedup 1.0000x reference)
"""Trainium2 Bass kernel for nn_ATConv (dynamic per-(b,c) 3x3 depthwise conv
between two 1x1 convs, with a pooled-gelu kernel-generation branch).

Sharding: data-parallel over batch B=16 across 8 NeuronCores (2 images/core).
Each core processes its 2 images as 3 "planes" of 128 partitions:
  P0 = img0 channels 0:128, P1 = img1 channels 0:128,
  P2 = packed [img0 c128:192 | img1 c128:192].

Per-core pipeline (all activations fp16, accumulation fp32 in PSUM/DVE):
  A: stream x (fp32 HBM -> fp16 SBUF via SWDGE cast), 1x1 conv Wx on PE,
     segment-pooling of x on DVE, PSUM->SBUF eviction (+bias, ->fp16) on ACT.
  B: kernel generation (tiny matmuls + erf-gelu + mean-subtract).
  C: depthwise 3x3 per (b,c): per tap, scaled shifted copy (tensor_scalar 4x,
     split across DVE/ACT/GPSIMD) then row-windowed tensor_tensor add (2x).
  D: 1x1 conv Wp on PE, eviction (+bias, fp32) on ACT, DMA out.
"""
import numpy as np

import concourse.bacc as bacc
import concourse.mybir as mybir
import concourse.tile as tile
from concourse import bass_utils

dt = mybir.dt
Alu = mybir.AluOpType
Act = mybir.ActivationFunctionType

B, C, H, W = 16, 192, 96, 96
L = H * W            # 9216
K2 = 9
SEG = L // K2        # 1024
NCORES = 8
NRANGE = L // SEG    # 9 ranges of 1024 for phase A
RT = 1024            # range tile width
INV_SQRT2 = float(1.0 / np.sqrt(2.0))

_BUILT = {}


def _conv_mms(nc, psums, lhsT_a, lhsT_b, xa0, xa1, xb, n0, n1):
    """Emit the 8 matmuls of one N-slice of the dual-image 1x1 conv.

    psums: dict with keys 'P0','P1','P2' -> PSUM APs [128, RT]
    lhsT_a: [128,192] (W.T rows 0:128), lhsT_b: [128,192] (W.T rows 128:192
    duplicated in both partition halves). xa0/xa1: [128,RT] fp16 (img c0:128),
    xb: [128,RT] packed (img0 c128:192 | img1 c128:192).
    """
    t = nc.tensor
    # img0 -> P0 (out c' 0:128)
    t.matmul(psums["P0"][:, n0:n1], lhsT_a[:, 0:128], xa0[:, n0:n1],
             start=True, stop=False)
    t.matmul(psums["P0"][:, n0:n1], lhsT_b[0:64, 0:128], xb[0:64, n0:n1],
             start=False, stop=True)
    # img1 -> P1
    t.matmul(psums["P1"][:, n0:n1], lhsT_a[:, 0:128], xa1[:, n0:n1],
             start=True, stop=False)
    t.matmul(psums["P1"][:, n0:n1], lhsT_b[64:128, 0:128], xb[64:128, n0:n1],
             start=False, stop=True)
    # img0 -> P2[0:64] (out c' 128:192)
    t.matmul(psums["P2"][0:64, n0:n1], lhsT_a[:, 128:192], xa0[:, n0:n1],
             start=True, stop=False)
    t.matmul(psums["P2"][0:64, n0:n1], lhsT_b[0:64, 128:192], xb[0:64, n0:n1],
             start=False, stop=True)
    # img1 -> P2[64:128]
    t.matmul(psums["P2"][64:128, n0:n1], lhsT_a[:, 128:192], xa1[:, n0:n1],
             start=True, stop=False, tile_position=(0, 64))
    t.matmul(psums["P2"][64:128, n0:n1], lhsT_b[64:128, 128:192],
             xb[64:128, n0:n1], start=False, stop=True, tile_position=(64, 64))


def build():
    nc = bacc.Bacc("TRN2", target_bir_lowering=False, debug=False,
                   num_devices=NCORES)

    # ---- DRAM tensors -------------------------------------------------
    x0 = nc.dram_tensor("x0", [C, L], dt.float32, kind="ExternalInput").ap()
    x1 = nc.dram_tensor("x1", [C, L], dt.float32, kind="ExternalInput").ap()
    wxT_a = nc.dram_tensor("wxT_a", [128, 192], dt.float16, kind="ExternalInput").ap()
    wxT_b = nc.dram_tensor("wxT_b", [128, 192], dt.float16, kind="ExternalInput").ap()
    wpT_a = nc.dram_tensor("wpT_a", [128, 192], dt.float16, kind="ExternalInput").ap()
    wpT_b = nc.dram_tensor("wpT_b", [128, 192], dt.float16, kind="ExternalInput").ap()
    wkT_a = nc.dram_tensor("wkT_a", [128, 192], dt.float16, kind="ExternalInput").ap()
    wkT_b = nc.dram_tensor("wkT_b", [128, 192], dt.float16, kind="ExternalInput").ap()
    wg2 = nc.dram_tensor("wg2", [9, 9], dt.float16, kind="ExternalInput").ap()
    bx_a = nc.dram_tensor("bx_a", [128, 1], dt.float32, kind="ExternalInput").ap()
    bx_b = nc.dram_tensor("bx_b", [128, 1], dt.float32, kind="ExternalInput").ap()
    bp_a = nc.dram_tensor("bp_a", [128, 1], dt.float32, kind="ExternalInput").ap()
    bp_b = nc.dram_tensor("bp_b", [128, 1], dt.float32, kind="ExternalInput").ap()
    dc_a = nc.dram_tensor("dc_a", [128, 1], dt.float32, kind="ExternalInput").ap()
    dc_b = nc.dram_tensor("dc_b", [128, 1], dt.float32, kind="ExternalInput").ap()
    bk_bc = nc.dram_tensor("bk_bc", [9, 192], dt.float32, kind="ExternalInput").ap()
    bg_bc = nc.dram_tensor("bg_bc", [128, 9], dt.float32, kind="ExternalInput").ap()
    out0 = nc.dram_tensor("out0", [C, L], dt.float32, kind="ExternalOutput").ap()
    out1 = nc.dram_tensor("out1", [C, L], dt.float32, kind="ExternalOutput").ap()
    dbg = {}
    if _BUILT.get("debug"):
        dbg["xpe"] = nc.dram_tensor("d_xpe", [128, L], dt.float16, kind="ExternalOutput").ap()
        dbg["y"] = nc.dram_tensor("d_y", [128, L], dt.float16, kind="ExternalOutput").ap()
        dbg["pool"] = nc.dram_tensor("d_pool", [128, 9], dt.float32, kind="ExternalOutput").ap()
        dbg["kfin"] = nc.dram_tensor("d_kfin", [128, 9], dt.float32, kind="ExternalOutput").ap()
        dbg["g"] = nc.dram_tensor("d_g", [9, 192], dt.float16, kind="ExternalOutput").ap()

    PL = ["P0", "P1", "P2"]

    with tile.TileContext(nc) as tc:
        with tc.tile_pool(name="wpool", bufs=1) as wp, \
             tc.tile_pool(name="xppool", bufs=1) as xpp, \
             tc.tile_pool(name="small", bufs=1) as sm:
            # ---- persistent weight/bias tiles ----
            wxa = wp.tile([128, 192], dt.float16, tag="wxa")
            wxb = wp.tile([128, 192], dt.float16, tag="wxb")
            wpa = wp.tile([128, 192], dt.float16, tag="wpa")
            wpb = wp.tile([128, 192], dt.float16, tag="wpb")
            wka = wp.tile([128, 192], dt.float16, tag="wka")
            wkb = wp.tile([128, 192], dt.float16, tag="wkb")
            wgt = wp.tile([9, 9], dt.float16, tag="wgt")
            for tl, src in [(wxa, wxT_a), (wxb, wxT_b), (wpa, wpT_a),
                            (wpb, wpT_b), (wka, wkT_a), (wkb, wkT_b),
                            (wgt, wg2)]:
                nc.sync.dma_start(tl[:], src[:, :])
            bias = {}
            for nm, src in [("bx_a", bx_a), ("bx_b", bx_b), ("bp_a", bp_a),
                            ("bp_b", bp_b), ("dc_a", dc_a), ("dc_b", dc_b)]:
                tl = wp.tile([128, 1], dt.float32, tag=nm)
                nc.sync.dma_start(tl[:], src[:, :])
                bias[nm] = tl
            bkb = wp.tile([9, 192], dt.float32, tag="bkb")
            nc.sync.dma_start(bkb[:], bk_bc[:, :])
            bgb = wp.tile([128, 9], dt.float32, tag="bgb")
            nc.sync.dma_start(bgb[:], bg_bc[:, :])

            # sigmoid(dc) per plane
            factor = {}
            for p, src in [("P0", "dc_a"), ("P1", "dc_a"), ("P2", "dc_b")]:
                if src == "dc_a" and p == "P1":
                    factor[p] = factor["P0"]
                    continue
                f = sm.tile([128, 1], dt.float32, tag=f"factor{p}", name=f"factor{p}")
                nc.scalar.activation(f[:], bias[src][:], Act.Sigmoid)
                factor[p] = f
            factor["P1"] = factor["P0"]

            # ---- xp planes (fp16), even + odd copies ----
            xpe = {p: xpp.tile([128, L], dt.float16, tag=f"xpe{p}", name=f"xpe{p}") for p in PL}

            # pool sums per plane
            pool = {p: sm.tile([128, 9], dt.float32, tag=f"pool{p}", name=f"pool{p}") for p in PL}

            biasx = {"P0": bias["bx_a"], "P1": bias["bx_a"], "P2": bias["bx_b"]}
            biasp = {"P0": bias["bp_a"], "P1": bias["bp_a"], "P2": bias["bp_b"]}

            # ================= PHASE A =================
            with tc.tile_pool(name="xring", bufs=3) as xr, \
                 tc.tile_pool(name="psA", bufs=1, space="PSUM") as psA:
                for r in range(NRANGE):
                    l0 = r * RT
                    xa0 = xr.tile([128, RT], dt.float16, tag="xa0")
                    xa1 = xr.tile([128, RT], dt.float16, tag="xa1")
                    xb = xr.tile([128, RT], dt.float16, tag="xb")
                    nc.gpsimd.dma_start(xa0[:], x0[0:128, l0:l0 + RT])
                    nc.gpsimd.dma_start(xa1[:], x1[0:128, l0:l0 + RT])
                    nc.gpsimd.dma_start(xb[0:64, :], x0[128:192, l0:l0 + RT])
                    nc.gpsimd.dma_start(xb[64:128, :], x1[128:192, l0:l0 + RT])

                    psums = {p: psA.tile([128, RT], dt.float32, tag=f"psA{p}", name=f"psA{p}")
                             for p in PL}
                    for (n0, n1) in [(0, 512), (512, 1024)]:
                        _conv_mms(nc, psums, wxa, wxb, xa0, xa1, xb, n0, n1)

                    # pooling: segment r sums (x is fp16; accumulate fp32)
                    nc.vector.tensor_reduce(pool["P0"][:, r:r + 1], xa0[:],
                                            mybir.AxisListType.X, Alu.add)
                    nc.vector.tensor_reduce(pool["P1"][:, r:r + 1], xa1[:],
                                            mybir.AxisListType.X, Alu.add)
                    nc.vector.tensor_reduce(pool["P2"][:, r:r + 1], xb[:],
                                            mybir.AxisListType.X, Alu.add)

                    # evict psum -> xp_even (+bx bias, cast fp16)
                    for p in PL:
                        nc.scalar.activation(xpe[p][:, l0:l0 + RT], psums[p][:],
                                             Act.Identity, bias=biasx[p][:])

            # ================= PHASE B (kernel generation) =================
            kfin = {}
            with tc.tile_pool(name="psB", bufs=1, space="PSUM") as psB:
                # scaled fp16 pool means
                pool16 = {}
                for p in PL:
                    t16 = sm.tile([128, 9], dt.float16, tag=f"pool16{p}", name=f"pool16{p}")
                    nc.vector.tensor_scalar(t16[:], pool[p][:], 1.0 / SEG, None,
                                            Alu.mult)
                    pool16[p] = t16
                # k1T = (Wk @ pool_mean).T per image: [9, 192]
                g16 = {}
                for i, (pa, lo, hi) in enumerate([("P0", 0, 64), ("P1", 64, 128)]):
                    k1 = psB.tile([9, 192], dt.float32, tag=f"k1T{i}", name=f"k1T{i}")
                    nc.tensor.matmul(k1[:], pool16[pa][:], wka[:],
                                     start=True, stop=False)
                    nc.tensor.matmul(k1[:], pool16["P2"][lo:hi, :],
                                     wkb[lo:hi, :], start=False, stop=True)
                    s = sm.tile([9, 192], dt.float32, tag=f"sB{i}", name=f"sB{i}")
                    nc.vector.tensor_tensor(s[:], k1[:], bkb[:], Alu.add)
                    e = sm.tile([9, 192], dt.float32, tag=f"eB{i}", name=f"eB{i}")
                    nc.scalar.activation(e[:], s[:], Act.Erf, scale=INV_SQRT2)
                    g = sm.tile([9, 192], dt.float16, tag=f"gB{i}", name=f"gB{i}")
                    nc.vector.scalar_tensor_tensor(g[:], e[:], 1.0, s[:],
                                                   Alu.add, Alu.mult)
                    g16[i] = g
                # k9 planes = g @ (0.5*Wg).T  (+bg, mean-subtract)
                k9ps = {}
                k9ps["P0"] = psB.tile([128, 9], dt.float32, tag="k9P0", name="k9P0")
                k9ps["P1"] = psB.tile([128, 9], dt.float32, tag="k9P1", name="k9P1")
                k9ps["P2"] = psB.tile([128, 9], dt.float32, tag="k9P2", name="k9P2")
                nc.tensor.matmul(k9ps["P0"][:], g16[0][:, 0:128], wgt[:],
                                 start=True, stop=True)
                nc.tensor.matmul(k9ps["P1"][:], g16[1][:, 0:128], wgt[:],
                                 start=True, stop=True)
                nc.tensor.matmul(k9ps["P2"][0:64, :], g16[0][:, 128:192], wgt[:],
                                 start=True, stop=True)
                nc.tensor.matmul(k9ps["P2"][64:128, :], g16[1][:, 128:192],
                                 wgt[:], start=True, stop=True,
                                 tile_position=(0, 64))
                for p in PL:
                    kb = sm.tile([128, 9], dt.float32, tag=f"kb{p}", name=f"kb{p}")
                    nc.vector.tensor_tensor(kb[:], k9ps[p][:], bgb[:], Alu.add)
                    ms = sm.tile([128, 1], dt.float32, tag=f"ms{p}", name=f"ms{p}")
                    nc.vector.tensor_reduce(ms[:], kb[:], mybir.AxisListType.X,
                                            Alu.add)
                    m2 = sm.tile([128, 1], dt.float32, tag=f"m2{p}", name=f"m2{p}")
                    nc.vector.tensor_scalar(m2[:], ms[:], factor[p][:], 1.0 / 9,
                                            Alu.mult, Alu.mult)
                    kf = sm.tile([128, 9], dt.float32, tag=f"kfin{p}", name=f"kfin{p}")
                    nc.vector.tensor_scalar(kf[:], kb[:], m2[:], None,
                                            Alu.subtract)
                    kfin[p] = kf

            if _BUILT.get("debug"):
                nc.sync.dma_start(dbg["pool"][:, :], pool["P0"][:])
                nc.sync.dma_start(dbg["kfin"][:, :], kfin["P0"][:])
                nc.sync.dma_start(dbg["g"][:, :], g16[0][:])
                nc.sync.dma_start(dbg["xpe"][:, :], xpe["P0"][:])
            # ================= PHASE C (depthwise) + D (Wp conv) ==========
            with tc.tile_pool(name="ypool", bufs=1) as yp, \
                 tc.tile_pool(name="tpool", bufs=2) as tp, \
                 tc.tile_pool(name="psD", bufs=1, space="PSUM") as psD, \
                 tc.tile_pool(name="stage", bufs=2) as stg:
                yt = {p: yp.tile([128, L], dt.float16, tag=f"y{p}", name=f"y{p}") for p in PL}
                # taps: (dh, dw) in 0..2 ; delta = (dh-1, dw-1); k col = 3*dh+dw
                # scaled-copy engine per tap: odd-dw shifts are 4B-misaligned
                # for DVE fast modes, so they go to ACT/GPSIMD.
                ts_engine = {1: "dve", 7: "dve",
                             0: "act", 2: "gps", 3: "act", 5: "gps",
                             6: "gps", 8: "act"}
                for p in ["P2", "P0", "P1"]:
                    y = yt[p]
                    y3 = y[:].rearrange("c (h w) -> c h w", h=H)
                    # center tap (dh=1, dw=1) initializes y
                    nc.vector.tensor_scalar(y[:], xpe[p][:],
                                            kfin[p][:, 4:5], None, Alu.mult)
                    for tap in [0, 1, 2, 3, 5, 6, 7, 8]:
                        dh, dw = tap // 3, tap % 3
                        ddh, ddw = dh - 1, dw - 1
                        ksc = kfin[p][:, tap:tap + 1]
                        t = tp.tile([128, L], dt.float16, tag="tscratch")
                        # t[p] = xp[p+ddw] over the valid sub-range
                        if ddw == 0:
                            src, dst = xpe[p][:, 0:L], t[:, 0:L]
                        elif ddw == 1:
                            src, dst = xpe[p][:, 1:L], t[:, 0:L - 1]
                        else:
                            src, dst = xpe[p][:, 0:L - 1], t[:, 1:L]
                        eng = ts_engine[tap]
                        if eng == "dve":
                            nc.vector.tensor_scalar(dst, src, ksc, None,
                                                    Alu.mult)
                        elif eng == "act":
                            nc.scalar.activation(dst, src, Act.Copy,
                                                 scale=ksc)
                        else:
                            nc.gpsimd.tensor_scalar(dst, src, ksc, None,
                                                    Alu.mult)
                        t3 = t[:].rearrange("c (h w) -> c h w", h=H)
                        # zero the wrapped (and unwritten corner) column
                        if ddw == 1:
                            nc.vector.memset(t3[:, :, W - 1:W], 0.0)
                        elif ddw == -1:
                            nc.vector.memset(t3[:, :, 0:1], 0.0)
                        # row-windowed accumulate: y[r0:r1] += t[r0+ddh:r1+ddh]
                        r0 = max(0, -ddh)
                        r1 = H - max(0, ddh)
                        nc.vector.tensor_tensor(y3[:, r0:r1, :],
                                                t3[:, r0 + ddh:r1 + ddh, :],
                                                y3[:, r0:r1, :], Alu.add)

                    # -------- PHASE D for this plane's dependents --------
                if _BUILT.get("debug"):
                    nc.sync.dma_start(dbg["y"][:, :], yt["P0"][:])
                # D: per image, after its y planes are done (P2 first, so
                # img0 ready after P0, img1 after P1)
                for r in range(NRANGE):
                    l0 = r * RT
                    psums = {p: psD.tile([128, RT], dt.float32, tag=f"psD{p}", name=f"psD{p}")
                             for p in PL}
                    for (n0, n1) in [(0, 512), (512, 1024)]:
                        _conv_mms(nc, psums, wpa, wpb,
                                  yt["P0"][:, l0:l0 + RT],
                                  yt["P1"][:, l0:l0 + RT],
                                  yt["P2"][:, l0:l0 + RT], n0, n1)
                    for p in PL:
                        st = stg.tile([128, RT], dt.float32, tag=f"st{p}", name=f"st{p}")
                        nc.scalar.activation(st[:], psums[p][:], Act.Identity,
                                             bias=biasp[p][:])
                        if p == "P0":
                            nc.sync.dma_start(out0[0:128, l0:l0 + RT], st[:])
                        elif p == "P1":
                            nc.sync.dma_start(out1[0:128, l0:l0 + RT], st[:])
                        else:
                            nc.sync.dma_start(out0[128:192, l0:l0 + RT],
                                              st[0:64, :])
                            nc.sync.dma_start(out1[128:192, l0:l0 + RT],
                                              st[64:128, :])

    nc.compile()
    return nc


def _get_nc():
    if "nc" not in _BUILT:
        _BUILT["nc"] = build()
    return _BUILT["nc"]


def kernel(x, Wk, bk, Wg, bg, Wx, bx, Wp, bp, dc):
    nc = _get_nc()
    x = np.asarray(x, dtype=np.float32)
    f32 = lambda a: np.ascontiguousarray(np.asarray(a, dtype=np.float32))
    f16T = lambda a: np.ascontiguousarray(
        np.asarray(a, dtype=np.float32).T.astype(np.float16))

    WxT = f16T(Wx)          # [c_in, c_out]
    WpT = f16T(Wp)
    WkT = f16T(Wk)
    wg2 = np.ascontiguousarray(
        (0.5 * np.asarray(Wg, dtype=np.float32)).T.astype(np.float16))
    dup = lambda wT: np.ascontiguousarray(
        np.concatenate([wT[128:192], wT[128:192]], axis=0))
    colv = lambda v, lo, hi: np.ascontiguousarray(
        np.asarray(v, dtype=np.float32)[lo:hi].reshape(-1, 1))
    dup_col = lambda v: np.ascontiguousarray(
        np.concatenate([colv(v, 128, 192), colv(v, 128, 192)], axis=0))

    shared = {
        "wxT_a": WxT[0:128], "wxT_b": dup(WxT),
        "wpT_a": WpT[0:128], "wpT_b": dup(WpT),
        "wkT_a": WkT[0:128], "wkT_b": dup(WkT),
        "wg2": wg2,
        "bx_a": colv(bx, 0, 128), "bx_b": dup_col(bx),
        "bp_a": colv(bp, 0, 128), "bp_b": dup_col(bp),
        "dc_a": colv(dc, 0, 128), "dc_b": dup_col(dc),
        "bk_bc": np.ascontiguousarray(
            np.tile(f32(bk).reshape(1, C), (9, 1))),
        "bg_bc": np.ascontiguousarray(
            np.tile(f32(bg).reshape(1, 9), (128, 1))),
    }
    in_maps = []
    for core in range(NCORES):
        m = dict(shared)
        m["x0"] = np.ascontiguousarray(x[2 * core].reshape(C, L))
        m["x1"] = np.ascontiguousarray(x[2 * core + 1].reshape(C, L))
        in_maps.append(m)

    res = bass_utils.run_bass_kernel_spmd(nc, in_maps,
                                          core_ids=list(range(NCORES)))
    out = np.empty((B, C, H, W), dtype=np.float32)
    for core in range(NCORES):
        out[2 * core] = res.results[core]["out0"].reshape(C, H, W)
        out[2 * core + 1] = res.results[core]["out1"].reshape(C, H, W)
    return out


# revision 7
# speedup vs baseline: 3.9903x; 3.9903x over previous
"""Trainium2 Bass kernel for nn_ATConv (dynamic per-(b,c) 3x3 depthwise conv
between two 1x1 convs, with a pooled-gelu kernel-generation branch).

Sharding: data-parallel over batch B=16 across 8 NeuronCores (2 images/core).
Each core processes its 2 images as 3 "planes" of 128 partitions:
  P0 = img0 channels 0:128, P1 = img1 channels 0:128,
  P2 = packed [img0 c128:192 | img1 c128:192].

Per-core pipeline (all activations fp16, accumulation fp32 in PSUM/DVE):
  A: stream x (fp32 HBM -> fp16 SBUF via SWDGE cast), 1x1 conv Wx on PE,
     segment-pooling of x on DVE, PSUM->SBUF eviction (+bias, ->fp16) on ACT.
  B: kernel generation (tiny matmuls + erf-gelu + mean-subtract).
  C: depthwise 3x3 per (b,c): per tap, scaled shifted copy (tensor_scalar 4x,
     split across DVE/ACT/GPSIMD) then row-windowed tensor_tensor add (2x).
  D: 1x1 conv Wp on PE, eviction (+bias, fp32) on ACT, DMA out.
"""
import numpy as np

import concourse.bacc as bacc
import concourse.mybir as mybir
import concourse.tile as tile
from concourse import bass_utils

dt = mybir.dt
Alu = mybir.AluOpType
Act = mybir.ActivationFunctionType

B, C, H, W = 16, 192, 96, 96
L = H * W            # 9216
K2 = 9
SEG = L // K2        # 1024
NCORES = 8
NRANGE = L // SEG    # 9 ranges of 1024 for phase A
RT = 1024            # range tile width
INV_SQRT2 = float(1.0 / np.sqrt(2.0))

_BUILT = {}


def _conv_mms(nc, psums, lhsT_a, lhsT_b, xa0, xa1, xb, n0, n1):
    """Emit the 8 matmuls of one N-slice of the dual-image 1x1 conv.

    psums: dict with keys 'P0','P1','P2' -> PSUM APs [128, RT]
    lhsT_a: [128,192] (W.T rows 0:128), lhsT_b: [128,192] (W.T rows 128:192
    duplicated in both partition halves). xa0/xa1: [128,RT] fp16 (img c0:128),
    xb: [128,RT] packed (img0 c128:192 | img1 c128:192).
    """
    t = nc.tensor
    # img0 -> P0 (out c' 0:128)
    t.matmul(psums["P0"][:, n0:n1], lhsT_a[:, 0:128], xa0[:, n0:n1],
             start=True, stop=False)
    t.matmul(psums["P0"][:, n0:n1], lhsT_b[0:64, 0:128], xb[0:64, n0:n1],
             start=False, stop=True)
    # img1 -> P1
    t.matmul(psums["P1"][:, n0:n1], lhsT_a[:, 0:128], xa1[:, n0:n1],
             start=True, stop=False)
    t.matmul(psums["P1"][:, n0:n1], lhsT_b[64:128, 0:128], xb[64:128, n0:n1],
             start=False, stop=True)
    # img0 -> P2[0:64] (out c' 128:192)
    t.matmul(psums["P2"][0:64, n0:n1], lhsT_a[:, 128:192], xa0[:, n0:n1],
             start=True, stop=False)
    t.matmul(psums["P2"][0:64, n0:n1], lhsT_b[0:64, 128:192], xb[0:64, n0:n1],
             start=False, stop=True)
    # img1 -> P2[64:128]
    t.matmul(psums["P2"][64:128, n0:n1], lhsT_a[:, 128:192], xa1[:, n0:n1],
             start=True, stop=False, tile_position=(0, 64))
    t.matmul(psums["P2"][64:128, n0:n1], lhsT_b[64:128, 128:192],
             xb[64:128, n0:n1], start=False, stop=True, tile_position=(64, 64))


def build():
    nc = bacc.Bacc("TRN2", target_bir_lowering=False, debug=False,
                   num_devices=NCORES)

    # ---- DRAM tensors -------------------------------------------------
    x0 = nc.dram_tensor("x0", [C, L], dt.float32, kind="ExternalInput").ap()
    x1 = nc.dram_tensor("x1", [C, L], dt.float32, kind="ExternalInput").ap()
    wxT_a = nc.dram_tensor("wxT_a", [128, 192], dt.float16, kind="ExternalInput").ap()
    wxT_b = nc.dram_tensor("wxT_b", [128, 192], dt.float16, kind="ExternalInput").ap()
    wpT_a = nc.dram_tensor("wpT_a", [128, 192], dt.float16, kind="ExternalInput").ap()
    wpT_b = nc.dram_tensor("wpT_b", [128, 192], dt.float16, kind="ExternalInput").ap()
    wkT_a = nc.dram_tensor("wkT_a", [128, 192], dt.float16, kind="ExternalInput").ap()
    wkT_b = nc.dram_tensor("wkT_b", [128, 192], dt.float16, kind="ExternalInput").ap()
    wg2 = nc.dram_tensor("wg2", [9, 9], dt.float16, kind="ExternalInput").ap()
    bx_a = nc.dram_tensor("bx_a", [128, 1], dt.float32, kind="ExternalInput").ap()
    bx_b = nc.dram_tensor("bx_b", [128, 1], dt.float32, kind="ExternalInput").ap()
    bp_a = nc.dram_tensor("bp_a", [128, 1], dt.float32, kind="ExternalInput").ap()
    bp_b = nc.dram_tensor("bp_b", [128, 1], dt.float32, kind="ExternalInput").ap()
    dc_a = nc.dram_tensor("dc_a", [128, 1], dt.float32, kind="ExternalInput").ap()
    dc_b = nc.dram_tensor("dc_b", [128, 1], dt.float32, kind="ExternalInput").ap()
    bk_bc = nc.dram_tensor("bk_bc", [9, 192], dt.float32, kind="ExternalInput").ap()
    bg_bc = nc.dram_tensor("bg_bc", [128, 9], dt.float32, kind="ExternalInput").ap()
    out0 = nc.dram_tensor("out0", [C, L], dt.float32, kind="ExternalOutput").ap()
    out1 = nc.dram_tensor("out1", [C, L], dt.float32, kind="ExternalOutput").ap()
    dbg = {}
    if _BUILT.get("debug"):
        dbg["xpe"] = nc.dram_tensor("d_xpe", [128, L], dt.float16, kind="ExternalOutput").ap()
        dbg["y"] = nc.dram_tensor("d_y", [128, L], dt.float16, kind="ExternalOutput").ap()
        dbg["pool"] = nc.dram_tensor("d_pool", [128, 9], dt.float32, kind="ExternalOutput").ap()
        dbg["kfin"] = nc.dram_tensor("d_kfin", [128, 9], dt.float32, kind="ExternalOutput").ap()
        dbg["g"] = nc.dram_tensor("d_g", [9, 192], dt.float16, kind="ExternalOutput").ap()

    PL = ["P0", "P1", "P2"]

    with tile.TileContext(nc) as tc:
        with tc.tile_pool(name="wpool", bufs=1) as wp, \
             tc.tile_pool(name="xppool", bufs=1) as xpp, \
             tc.tile_pool(name="small", bufs=1) as sm:
            # ---- persistent weight/bias tiles ----
            wxa = wp.tile([128, 192], dt.float16, tag="wxa")
            wxb = wp.tile([128, 192], dt.float16, tag="wxb")
            wpa = wp.tile([128, 192], dt.float16, tag="wpa")
            wpb = wp.tile([128, 192], dt.float16, tag="wpb")
            wka = wp.tile([128, 192], dt.float16, tag="wka")
            wkb = wp.tile([128, 192], dt.float16, tag="wkb")
            wgt = wp.tile([9, 9], dt.float16, tag="wgt")
            for tl, src in [(wxa, wxT_a), (wxb, wxT_b), (wpa, wpT_a),
                            (wpb, wpT_b), (wka, wkT_a), (wkb, wkT_b),
                            (wgt, wg2)]:
                nc.sync.dma_start(tl[:], src[:, :])
            bias = {}
            for nm, src in [("bx_a", bx_a), ("bx_b", bx_b), ("bp_a", bp_a),
                            ("bp_b", bp_b), ("dc_a", dc_a), ("dc_b", dc_b)]:
                tl = wp.tile([128, 1], dt.float32, tag=nm)
                nc.sync.dma_start(tl[:], src[:, :])
                bias[nm] = tl
            bkb = wp.tile([9, 192], dt.float32, tag="bkb")
            nc.sync.dma_start(bkb[:], bk_bc[:, :])
            bgb = wp.tile([128, 9], dt.float32, tag="bgb")
            nc.sync.dma_start(bgb[:], bg_bc[:, :])

            # sigmoid(dc) per plane
            factor = {}
            for p, src in [("P0", "dc_a"), ("P1", "dc_a"), ("P2", "dc_b")]:
                if src == "dc_a" and p == "P1":
                    factor[p] = factor["P0"]
                    continue
                f = sm.tile([128, 1], dt.float32, tag=f"factor{p}", name=f"factor{p}")
                nc.scalar.activation(f[:], bias[src][:], Act.Sigmoid)
                factor[p] = f
            factor["P1"] = factor["P0"]

            # ---- xp planes (fp16), even + odd copies ----
            xpe = {p: xpp.tile([128, L], dt.float16, tag=f"xpe{p}", name=f"xpe{p}") for p in PL}

            # pool sums per plane
            pool = {p: sm.tile([128, 9], dt.float32, tag=f"pool{p}", name=f"pool{p}") for p in PL}

            biasx = {"P0": bias["bx_a"], "P1": bias["bx_a"], "P2": bias["bx_b"]}
            biasp = {"P0": bias["bp_a"], "P1": bias["bp_a"], "P2": bias["bp_b"]}

            # ================= PHASE A =================
            with tc.tile_pool(name="xring", bufs=3) as xr, \
                 tc.tile_pool(name="psA", bufs=1, space="PSUM") as psA:
                for r in range(NRANGE):
                    l0 = r * RT
                    xa0 = xr.tile([128, RT], dt.float16, tag="xa0")
                    xa1 = xr.tile([128, RT], dt.float16, tag="xa1")
                    xb = xr.tile([128, RT], dt.float16, tag="xb")
                    nc.gpsimd.dma_start(xa0[:], x0[0:128, l0:l0 + RT])
                    nc.gpsimd.dma_start(xa1[:], x1[0:128, l0:l0 + RT])
                    nc.gpsimd.dma_start(xb[0:64, :], x0[128:192, l0:l0 + RT])
                    nc.gpsimd.dma_start(xb[64:128, :], x1[128:192, l0:l0 + RT])

                    psums = {p: psA.tile([128, RT], dt.float32, tag=f"psA{p}", name=f"psA{p}")
                             for p in PL}
                    for (n0, n1) in [(0, 512), (512, 1024)]:
                        _conv_mms(nc, psums, wxa, wxb, xa0, xa1, xb, n0, n1)

                    # pooling: segment r sums (x is fp16; accumulate fp32)
                    nc.vector.tensor_reduce(pool["P0"][:, r:r + 1], xa0[:],
                                            mybir.AxisListType.X, Alu.add)
                    nc.vector.tensor_reduce(pool["P1"][:, r:r + 1], xa1[:],
                                            mybir.AxisListType.X, Alu.add)
                    nc.vector.tensor_reduce(pool["P2"][:, r:r + 1], xb[:],
                                            mybir.AxisListType.X, Alu.add)

                    # evict psum -> xp_even (+bx bias, cast fp16)
                    for p in PL:
                        nc.scalar.activation(xpe[p][:, l0:l0 + RT], psums[p][:],
                                             Act.Identity, bias=biasx[p][:])

            # ================= PHASE B (kernel generation) =================
            kfin = {}
            with tc.tile_pool(name="psB", bufs=1, space="PSUM") as psB:
                # scaled fp16 pool means
                pool16 = {}
                for p in PL:
                    t16 = sm.tile([128, 9], dt.float16, tag=f"pool16{p}", name=f"pool16{p}")
                    nc.vector.tensor_scalar(t16[:], pool[p][:], 1.0 / SEG, None,
                                            Alu.mult)
                    pool16[p] = t16
                # k1T = (Wk @ pool_mean).T per image: [9, 192]
                g16 = {}
                for i, (pa, lo, hi) in enumerate([("P0", 0, 64), ("P1", 64, 128)]):
                    k1 = psB.tile([9, 192], dt.float32, tag=f"k1T{i}", name=f"k1T{i}")
                    nc.tensor.matmul(k1[:], pool16[pa][:], wka[:],
                                     start=True, stop=False)
                    nc.tensor.matmul(k1[:], pool16["P2"][lo:hi, :],
                                     wkb[lo:hi, :], start=False, stop=True)
                    s = sm.tile([9, 192], dt.float32, tag=f"sB{i}", name=f"sB{i}")
                    nc.vector.tensor_tensor(s[:], k1[:], bkb[:], Alu.add)
                    e = sm.tile([9, 192], dt.float32, tag=f"eB{i}", name=f"eB{i}")
                    nc.scalar.activation(e[:], s[:], Act.Erf, scale=INV_SQRT2)
                    g = sm.tile([9, 192], dt.float16, tag=f"gB{i}", name=f"gB{i}")
                    nc.vector.scalar_tensor_tensor(g[:], e[:], 1.0, s[:],
                                                   Alu.add, Alu.mult)
                    g16[i] = g
                # k9 planes = g @ (0.5*Wg).T  (+bg, mean-subtract)
                k9ps = {}
                k9ps["P0"] = psB.tile([128, 9], dt.float32, tag="k9P0", name="k9P0")
                k9ps["P1"] = psB.tile([128, 9], dt.float32, tag="k9P1", name="k9P1")
                k9ps["P2"] = psB.tile([128, 9], dt.float32, tag="k9P2", name="k9P2")
                nc.tensor.matmul(k9ps["P0"][:], g16[0][:, 0:128], wgt[:],
                                 start=True, stop=True)
                nc.tensor.matmul(k9ps["P1"][:], g16[1][:, 0:128], wgt[:],
                                 start=True, stop=True)
                nc.tensor.matmul(k9ps["P2"][0:64, :], g16[0][:, 128:192], wgt[:],
                                 start=True, stop=True)
                nc.tensor.matmul(k9ps["P2"][64:128, :], g16[1][:, 128:192],
                                 wgt[:], start=True, stop=True,
                                 tile_position=(0, 64))
                for p in PL:
                    kb = sm.tile([128, 9], dt.float32, tag=f"kb{p}", name=f"kb{p}")
                    nc.vector.tensor_tensor(kb[:], k9ps[p][:], bgb[:], Alu.add)
                    ms = sm.tile([128, 1], dt.float32, tag=f"ms{p}", name=f"ms{p}")
                    nc.vector.tensor_reduce(ms[:], kb[:], mybir.AxisListType.X,
                                            Alu.add)
                    m2 = sm.tile([128, 1], dt.float32, tag=f"m2{p}", name=f"m2{p}")
                    nc.vector.tensor_scalar(m2[:], ms[:], factor[p][:], 1.0 / 9,
                                            Alu.mult, Alu.mult)
                    kf = sm.tile([128, 9], dt.float32, tag=f"kfin{p}", name=f"kfin{p}")
                    nc.vector.tensor_scalar(kf[:], kb[:], m2[:], None,
                                            Alu.subtract)
                    kfin[p] = kf

            if _BUILT.get("debug"):
                nc.sync.dma_start(dbg["pool"][:, :], pool["P0"][:])
                nc.sync.dma_start(dbg["kfin"][:, :], kfin["P0"][:])
                nc.sync.dma_start(dbg["g"][:, :], g16[0][:])
                nc.sync.dma_start(dbg["xpe"][:, :], xpe["P0"][:])
            # ================= PHASE C (depthwise) + D (Wp conv) ==========
            with tc.tile_pool(name="ypool", bufs=1) as yp, \
                 tc.tile_pool(name="tpool", bufs=2) as tp, \
                 tc.tile_pool(name="xopool", bufs=1) as xop, \
                 tc.tile_pool(name="psD", bufs=1, space="PSUM") as psD, \
                 tc.tile_pool(name="stage", bufs=1) as stg:
                yt = {p: yp.tile([128, L], dt.float16, tag=f"y{p}", name=f"y{p}") for p in PL}
                # taps: (dh, dw) in 0..2 ; delta = (dh-1, dw-1); k col = 3*dh+dw
                # Scaled shifted copies read xpe (ddw=0) or the +1-shifted
                # copy xpo (ddw=+-1) so DVE reads stay 4B-aligned (fast mode).
                ts_engine = {1: "dve", 7: "dve", 3: "dve",
                             0: "act", 2: "act", 5: "act", 6: "act", 8: "act"}
                for p in ["P2", "P0", "P1"]:
                    y = yt[p]
                    # xpo[i] = xpe[i-1]
                    xpo = xop.tile([128, L + 4], dt.float16, tag="xpo",
                                   name="xpo")
                    nc.sync.dma_start(xpo[:, 1:1 + L], xpe[p][:, 0:L])
                    # center tap (dh=1, dw=1) initializes y
                    nc.vector.tensor_scalar(y[:], xpe[p][:],
                                            kfin[p][:, 4:5], None, Alu.mult)
                    for tap in [0, 1, 2, 3, 5, 6, 7, 8]:
                        dh, dw = tap // 3, tap % 3
                        ddh, ddw = dh - 1, dw - 1
                        ksc = kfin[p][:, tap:tap + 1]
                        t = tp.tile([128, L], dt.float16, tag="tscratch")
                        # t[i] = xp[i+ddw] (wrap columns cleaned below)
                        if ddw == 0:
                            src = xpe[p][:, 0:L]
                        elif ddw == 1:
                            src = xpo[:, 2:2 + L]
                        else:
                            src = xpo[:, 0:L]
                        if ts_engine[tap] == "dve":
                            nc.vector.tensor_scalar(t[:], src, ksc, None,
                                                    Alu.mult)
                        else:
                            nc.scalar.activation(t[:], src, Act.Copy,
                                                 scale=ksc)
                        t3 = t[:].rearrange("c (h w) -> c h w", h=H)
                        # zero the wrapped column
                        if ddw == 1:
                            nc.vector.memset(t3[:, :, W - 1:W], 0.0)
                        elif ddw == -1:
                            nc.vector.memset(t3[:, :, 0:1], 0.0)
                        # row-windowed accumulate, flat 2D APs:
                        # y[:, r0*W:r1*W] += t[:, (r0+ddh)*W:(r1+ddh)*W]
                        r0 = max(0, -ddh)
                        r1 = H - max(0, ddh)
                        nc.vector.tensor_tensor(
                            y[:, r0 * W:r1 * W],
                            t[:, (r0 + ddh) * W:(r1 + ddh) * W],
                            y[:, r0 * W:r1 * W], Alu.add)

                    # -------- PHASE D for this plane's dependents --------
                if _BUILT.get("debug"):
                    nc.sync.dma_start(dbg["y"][:, :], yt["P0"][:])
                # D: per image, after its y planes are done (P2 first, so
                # img0 ready after P0, img1 after P1)
                for r in range(NRANGE):
                    l0 = r * RT
                    psums = {p: psD.tile([128, RT], dt.float32, tag=f"psD{p}", name=f"psD{p}")
                             for p in PL}
                    for (n0, n1) in [(0, 512), (512, 1024)]:
                        _conv_mms(nc, psums, wpa, wpb,
                                  yt["P0"][:, l0:l0 + RT],
                                  yt["P1"][:, l0:l0 + RT],
                                  yt["P2"][:, l0:l0 + RT], n0, n1)
                    for p in PL:
                        st = stg.tile([128, RT], dt.float32, tag=f"st{p}", name=f"st{p}")
                        nc.scalar.activation(st[:], psums[p][:], Act.Identity,
                                             bias=biasp[p][:])
                        if p == "P0":
                            nc.sync.dma_start(out0[0:128, l0:l0 + RT], st[:])
                        elif p == "P1":
                            nc.sync.dma_start(out1[0:128, l0:l0 + RT], st[:])
                        else:
                            nc.sync.dma_start(out0[128:192, l0:l0 + RT],
                                              st[0:64, :])
                            nc.sync.dma_start(out1[128:192, l0:l0 + RT],
                                              st[64:128, :])

    nc.compile()
    return nc


def _get_nc():
    if "nc" not in _BUILT:
        _BUILT["nc"] = build()
    return _BUILT["nc"]


def kernel(x, Wk, bk, Wg, bg, Wx, bx, Wp, bp, dc):
    nc = _get_nc()
    x = np.asarray(x, dtype=np.float32)
    f32 = lambda a: np.ascontiguousarray(np.asarray(a, dtype=np.float32))
    f16T = lambda a: np.ascontiguousarray(
        np.asarray(a, dtype=np.float32).T.astype(np.float16))

    WxT = f16T(Wx)          # [c_in, c_out]
    WpT = f16T(Wp)
    WkT = f16T(Wk)
    wg2 = np.ascontiguousarray(
        (0.5 * np.asarray(Wg, dtype=np.float32)).T.astype(np.float16))
    dup = lambda wT: np.ascontiguousarray(
        np.concatenate([wT[128:192], wT[128:192]], axis=0))
    colv = lambda v, lo, hi: np.ascontiguousarray(
        np.asarray(v, dtype=np.float32)[lo:hi].reshape(-1, 1))
    dup_col = lambda v: np.ascontiguousarray(
        np.concatenate([colv(v, 128, 192), colv(v, 128, 192)], axis=0))

    shared = {
        "wxT_a": WxT[0:128], "wxT_b": dup(WxT),
        "wpT_a": WpT[0:128], "wpT_b": dup(WpT),
        "wkT_a": WkT[0:128], "wkT_b": dup(WkT),
        "wg2": wg2,
        "bx_a": colv(bx, 0, 128), "bx_b": dup_col(bx),
        "bp_a": colv(bp, 0, 128), "bp_b": dup_col(bp),
        "dc_a": colv(dc, 0, 128), "dc_b": dup_col(dc),
        "bk_bc": np.ascontiguousarray(
            np.tile(f32(bk).reshape(1, C), (9, 1))),
        "bg_bc": np.ascontiguousarray(
            np.tile(f32(bg).reshape(1, 9), (128, 1))),
    }
    in_maps = []
    for core in range(NCORES):
        m = dict(shared)
        m["x0"] = np.ascontiguousarray(x[2 * core].reshape(C, L))
        m["x1"] = np.ascontiguousarray(x[2 * core + 1].reshape(C, L))
        in_maps.append(m)

    res = bass_utils.run_bass_kernel_spmd(nc, in_maps,
                                          core_ids=list(range(NCORES)))
    out = np.empty((B, C, H, W), dtype=np.float32)
    for core in range(NCORES):
        out[2 * core] = res.results[core]["out0"].reshape(C, H, W)
        out[2 * core + 1] = res.results[core]["out1"].reshape(C, H, W)
    return out


# revision 10
# speedup vs baseline: 4.3984x; 1.1023x over previous
"""Trainium2 Bass kernel for nn_ATConv (dynamic per-(b,c) 3x3 depthwise conv
between two 1x1 convs, with a pooled-gelu kernel-generation branch).

Sharding: data-parallel over batch B=16 across 8 NeuronCores (2 images/core).
Each core processes its 2 images as 3 "planes" of 128 partitions:
  P0 = img0 channels 0:128, P1 = img1 channels 0:128,
  P2 = packed [img0 c128:192 | img1 c128:192].

Per-core pipeline:
  A: stream x (fp32 HBM, HWDGE), 1x1 conv Wx in fp32r on PE, segment pooling
     of x on DVE, PSUM->SBUF eviction (+bias, cast fp16) on ACT.
  B: kernel generation (tiny fp16 matmuls + erf-gelu + mean-subtract).
  C: depthwise 3x3 per (b,c) in fp16: per tap, a scaled shifted full-plane
     copy (tensor_scalar 4x on DVE / activation on ACT) + wrap-column memset,
     then a flat row-windowed tensor_tensor accumulate (2x on DVE).
  D: 1x1 conv Wp in fp16 on PE (split per image to overlap with C),
     eviction (+bias, fp32) on ACT, DMA out.
"""
import numpy as np

import concourse.bacc as bacc
import concourse.mybir as mybir
import concourse.tile as tile
from concourse import bass_utils

dt = mybir.dt
Alu = mybir.AluOpType
Act = mybir.ActivationFunctionType

B, C, H, W = 16, 192, 96, 96
L = H * W            # 9216
K2 = 9
SEG = L // K2        # 1024
NCORES = 8
NRANGE = L // SEG    # 9
RT = 1024
INV_SQRT2 = float(1.0 / np.sqrt(2.0))

_BUILT = {}


def _img_mms(nc, ps_a, ps2, half, lhsT_a, lhsT_b, xa, xb, n0, n1):
    """One image's matmuls for one N-slice of a dual-chunk 1x1 conv.

    ps_a: PSUM [128, RT] for out channels 0:128; ps2: PSUM [128, RT] whose
    `half` half holds out channels 128:192. xa: [128, RT] rhs (c 0:128),
    xb: [128, RT] packed rhs (its `half` half is this image's c 128:192).
    """
    t = nc.tensor
    lo, hi = (0, 64) if half == 0 else (64, 128)
    cpos = 0 if half == 0 else 64
    t.matmul(ps_a[:, n0:n1], lhsT_a[:, 0:128], xa[:, n0:n1],
             start=True, stop=False)
    t.matmul(ps_a[:, n0:n1], lhsT_b[lo:hi, 0:128], xb[lo:hi, n0:n1],
             start=False, stop=True)
    t.matmul(ps2[lo:hi, n0:n1], lhsT_a[:, 128:192], xa[:, n0:n1],
             start=True, stop=False,
             tile_position=(0, cpos) if cpos else None)
    t.matmul(ps2[lo:hi, n0:n1], lhsT_b[lo:hi, 128:192], xb[lo:hi, n0:n1],
             start=False, stop=True,
             tile_position=(lo, cpos) if cpos else None)


def build():
    nc = bacc.Bacc("TRN2", target_bir_lowering=False, debug=False,
                   num_devices=NCORES)

    # ---- DRAM tensors -------------------------------------------------
    f32r, f16, f32 = dt.float32r, dt.float16, dt.float32
    x0 = nc.dram_tensor("x0", [C, L], f32, kind="ExternalInput").ap()
    x1 = nc.dram_tensor("x1", [C, L], f32, kind="ExternalInput").ap()
    wxT_a = nc.dram_tensor("wxT_a", [128, 192], f16, kind="ExternalInput").ap()
    wxT_b = nc.dram_tensor("wxT_b", [128, 192], f16, kind="ExternalInput").ap()
    wpT_a = nc.dram_tensor("wpT_a", [128, 192], f16, kind="ExternalInput").ap()
    wpT_b = nc.dram_tensor("wpT_b", [128, 192], f16, kind="ExternalInput").ap()
    wkT_a = nc.dram_tensor("wkT_a", [128, 192], f16, kind="ExternalInput").ap()
    wkT_b = nc.dram_tensor("wkT_b", [128, 192], f16, kind="ExternalInput").ap()
    wg2 = nc.dram_tensor("wg2", [9, 9], f16, kind="ExternalInput").ap()
    bx_a = nc.dram_tensor("bx_a", [128, 1], f32, kind="ExternalInput").ap()
    bx_b = nc.dram_tensor("bx_b", [128, 1], f32, kind="ExternalInput").ap()
    bp_a = nc.dram_tensor("bp_a", [128, 1], f32, kind="ExternalInput").ap()
    bp_b = nc.dram_tensor("bp_b", [128, 1], f32, kind="ExternalInput").ap()
    dc_a = nc.dram_tensor("dc_a", [128, 1], f32, kind="ExternalInput").ap()
    dc_b = nc.dram_tensor("dc_b", [128, 1], f32, kind="ExternalInput").ap()
    bk_bc = nc.dram_tensor("bk_bc", [9, 192], f32, kind="ExternalInput").ap()
    bg_bc = nc.dram_tensor("bg_bc", [128, 9], f32, kind="ExternalInput").ap()
    out0 = nc.dram_tensor("out0", [C, L], f32, kind="ExternalOutput").ap()
    out1 = nc.dram_tensor("out1", [C, L], f32, kind="ExternalOutput").ap()

    PL = ["P0", "P1", "P2"]

    with tile.TileContext(nc) as tc:
        with tc.tile_pool(name="wpool", bufs=1) as wp, \
             tc.tile_pool(name="xppool", bufs=1) as xpp, \
             tc.tile_pool(name="small", bufs=1) as sm:
            # ---- persistent weight/bias tiles ----
            wxa = wp.tile([128, 192], f16, tag="wxa")
            wxb = wp.tile([128, 192], f16, tag="wxb")
            wpa = wp.tile([128, 192], f16, tag="wpa")
            wpb = wp.tile([128, 192], f16, tag="wpb")
            wka = wp.tile([128, 192], f16, tag="wka")
            wkb = wp.tile([128, 192], f16, tag="wkb")
            wgt = wp.tile([9, 9], f16, tag="wgt")
            for tl, src in [(wxa, wxT_a), (wxb, wxT_b), (wpa, wpT_a),
                            (wpb, wpT_b), (wka, wkT_a), (wkb, wkT_b),
                            (wgt, wg2)]:
                nc.sync.dma_start(tl[:], src[:, :])
            bias = {}
            for nm, src in [("bx_a", bx_a), ("bx_b", bx_b), ("bp_a", bp_a),
                            ("bp_b", bp_b), ("dc_a", dc_a), ("dc_b", dc_b)]:
                tl = wp.tile([128, 1], f32, tag=nm)
                nc.sync.dma_start(tl[:], src[:, :])
                bias[nm] = tl
            bkb = wp.tile([9, 192], f32, tag="bkb")
            nc.sync.dma_start(bkb[:], bk_bc[:, :])
            bgb = wp.tile([128, 9], f32, tag="bgb")
            nc.sync.dma_start(bgb[:], bg_bc[:, :])

            factor = {}
            for p, src in [("P0", "dc_a"), ("P2", "dc_b")]:
                f = sm.tile([128, 1], f32, tag=f"factor{p}", name=f"factor{p}")
                nc.scalar.activation(f[:], bias[src][:], Act.Sigmoid)
                factor[p] = f
            factor["P1"] = factor["P0"]

            xpe = {p: xpp.tile([128, L], f16, tag=f"xpe{p}", name=f"xpe{p}")
                   for p in PL}
            pool = {p: sm.tile([128, 9], f32, tag=f"pool{p}", name=f"pool{p}")
                    for p in PL}
            biasx = {"P0": bias["bx_a"], "P1": bias["bx_a"], "P2": bias["bx_b"]}
            biasp = {"P0": bias["bp_a"], "P1": bias["bp_a"], "P2": bias["bp_b"]}

            # ================= PHASE A =================
            with tc.tile_pool(name="xring", bufs=4) as xr, \
                 tc.tile_pool(name="psA", bufs=1, space="PSUM") as psA:
                for r in range(NRANGE):
                    l0 = r * RT
                    xa0 = xr.tile([128, RT], f16, tag="xa0")
                    xa1 = xr.tile([128, RT], f16, tag="xa1")
                    xb = xr.tile([128, RT], f16, tag="xb")
                    nc.gpsimd.dma_start(xa0[:], x0[0:128, l0:l0 + RT])
                    nc.gpsimd.dma_start(xa1[:], x1[0:128, l0:l0 + RT])
                    nc.gpsimd.dma_start(xb[0:64, :], x0[128:192, l0:l0 + RT])
                    nc.gpsimd.dma_start(xb[64:128, :], x1[128:192, l0:l0 + RT])

                    psums = {p: psA.tile([128, RT], f32, tag=f"psA{p}",
                                         name=f"psA{p}") for p in PL}
                    for (n0, n1) in [(0, 512), (512, 1024)]:
                        _img_mms(nc, psums["P0"], psums["P2"], 0, wxa, wxb,
                                 xa0, xb, n0, n1)
                        _img_mms(nc, psums["P1"], psums["P2"], 1, wxa, wxb,
                                 xa1, xb, n0, n1)

                    nc.vector.tensor_reduce(pool["P0"][:, r:r + 1], xa0[:],
                                            mybir.AxisListType.X, Alu.add)
                    nc.vector.tensor_reduce(pool["P1"][:, r:r + 1], xa1[:],
                                            mybir.AxisListType.X, Alu.add)
                    nc.vector.tensor_reduce(pool["P2"][:, r:r + 1], xb[:],
                                            mybir.AxisListType.X, Alu.add)

                    for p in PL:
                        nc.scalar.activation(xpe[p][:, l0:l0 + RT], psums[p][:],
                                             Act.Identity, bias=biasx[p][:])

            # ================= PHASE B (kernel generation) =================
            kfin = {}
            with tc.tile_pool(name="psB", bufs=1, space="PSUM") as psB:
                pool16 = {}
                for p in PL:
                    t16 = sm.tile([128, 9], f16, tag=f"pool16{p}",
                                  name=f"pool16{p}")
                    nc.vector.tensor_scalar(t16[:], pool[p][:], 1.0 / SEG,
                                            None, Alu.mult)
                    pool16[p] = t16
                g16 = {}
                for i, (pa, lo, hi) in enumerate([("P0", 0, 64),
                                                  ("P1", 64, 128)]):
                    k1 = psB.tile([9, 192], f32, tag=f"k1T{i}", name=f"k1T{i}")
                    nc.tensor.matmul(k1[:], pool16[pa][:], wka[:],
                                     start=True, stop=False)
                    nc.tensor.matmul(k1[:], pool16["P2"][lo:hi, :],
                                     wkb[lo:hi, :], start=False, stop=True)
                    s = sm.tile([9, 192], f32, tag=f"sB{i}", name=f"sB{i}")
                    nc.vector.tensor_tensor(s[:], k1[:], bkb[:], Alu.add)
                    e = sm.tile([9, 192], f32, tag=f"eB{i}", name=f"eB{i}")
                    nc.scalar.activation(e[:], s[:], Act.Erf, scale=INV_SQRT2)
                    g = sm.tile([9, 192], f16, tag=f"gB{i}", name=f"gB{i}")
                    nc.vector.scalar_tensor_tensor(g[:], e[:], 1.0, s[:],
                                                   Alu.add, Alu.mult)
                    g16[i] = g
                k9ps = {}
                for p in PL:
                    k9ps[p] = psB.tile([128, 9], f32, tag=f"k9{p}",
                                       name=f"k9{p}")
                nc.tensor.matmul(k9ps["P0"][:], g16[0][:, 0:128], wgt[:],
                                 start=True, stop=True)
                nc.tensor.matmul(k9ps["P1"][:], g16[1][:, 0:128], wgt[:],
                                 start=True, stop=True)
                nc.tensor.matmul(k9ps["P2"][0:64, :], g16[0][:, 128:192],
                                 wgt[:], start=True, stop=True)
                nc.tensor.matmul(k9ps["P2"][64:128, :], g16[1][:, 128:192],
                                 wgt[:], start=True, stop=True,
                                 tile_position=(0, 64))
                for p in PL:
                    kb = sm.tile([128, 9], f32, tag=f"kb{p}", name=f"kb{p}")
                    nc.vector.tensor_tensor(kb[:], k9ps[p][:], bgb[:], Alu.add)
                    ms = sm.tile([128, 1], f32, tag=f"ms{p}", name=f"ms{p}")
                    nc.vector.tensor_reduce(ms[:], kb[:],
                                            mybir.AxisListType.X, Alu.add)
                    m2 = sm.tile([128, 1], f32, tag=f"m2{p}", name=f"m2{p}")
                    nc.vector.tensor_scalar(m2[:], ms[:], factor[p][:],
                                            1.0 / 9, Alu.mult, Alu.mult)
                    kf = sm.tile([128, 9], f32, tag=f"kfin{p}",
                                 name=f"kfin{p}")
                    nc.vector.tensor_scalar(kf[:], kb[:], m2[:], None,
                                            Alu.subtract)
                    kfin[p] = kf

            # ========== PHASE C (depthwise) interleaved with PHASE D ======
            with tc.tile_pool(name="ypool", bufs=1) as yp, \
                 tc.tile_pool(name="tpool", bufs=3) as tp, \
                 tc.tile_pool(name="xopool", bufs=1) as xop, \
                 tc.tile_pool(name="psD", bufs=1, space="PSUM") as psD, \
                 tc.tile_pool(name="stage", bufs=2) as stg:
                yt = {p: yp.tile([128, L], f16, tag=f"y{p}", name=f"y{p}")
                      for p in PL}
                ts_engine = {1: "dve", 7: "dve", 3: "dve",
                             0: "act", 2: "act", 5: "act", 6: "act", 8: "act"}

                def phase_c(p):
                    y = yt[p]
                    xpo = xop.tile([128, L + 4], f16, tag="xpo", name="xpo")
                    nc.sync.dma_start(xpo[:, 1:1 + L], xpe[p][:, 0:L])
                    nc.vector.tensor_scalar(y[:], xpe[p][:],
                                            kfin[p][:, 4:5], None, Alu.mult)
                    for tap in [0, 1, 2, 3, 5, 6, 7, 8]:
                        dh, dw = tap // 3, tap % 3
                        ddh, ddw = dh - 1, dw - 1
                        ksc = kfin[p][:, tap:tap + 1]
                        t = tp.tile([128, L], f16, tag="tscratch",
                                    name="tscratch")
                        if ddw == 0:
                            src = xpe[p][:, 0:L]
                        elif ddw == 1:
                            src = xpo[:, 2:2 + L]
                        else:
                            src = xpo[:, 0:L]
                        if ts_engine[tap] == "dve":
                            nc.vector.tensor_scalar(t[:], src, ksc, None,
                                                    Alu.mult)
                        else:
                            nc.scalar.activation(t[:], src, Act.Copy,
                                                 scale=ksc)
                        t3 = t[:].rearrange("c (h w) -> c h w", h=H)
                        if ddw == 1:
                            nc.vector.memset(t3[:, :, W - 1:W], 0.0)
                        elif ddw == -1:
                            nc.vector.memset(t3[:, :, 0:1], 0.0)
                        r0 = max(0, -ddh)
                        r1 = H - max(0, ddh)
                        nc.vector.tensor_tensor(
                            y[:, r0 * W:r1 * W],
                            t[:, (r0 + ddh) * W:(r1 + ddh) * W],
                            y[:, r0 * W:r1 * W], Alu.add)

                def phase_d(img):
                    ya = yt["P0"] if img == 0 else yt["P1"]
                    out = out0 if img == 0 else out1
                    lo, hi = (0, 64) if img == 0 else (64, 128)
                    for r in range(NRANGE):
                        l0 = r * RT
                        pa = psD.tile([128, RT], f32, tag=f"psDa{img}",
                                      name=f"psDa{img}")
                        p2 = psD.tile([128, RT], f32, tag=f"psDb{img}",
                                      name=f"psDb{img}")
                        for (n0, n1) in [(0, 512), (512, 1024)]:
                            _img_mms(nc, pa, p2, img, wpa, wpb,
                                     ya[:, l0:l0 + RT], yt["P2"][:, l0:l0 + RT],
                                     n0, n1)
                        sta = stg.tile([128, RT], f32, tag="sta", name="sta")
                        nc.scalar.activation(sta[:], pa[:], Act.Identity,
                                             bias=biasp["P0"][:])
                        nc.sync.dma_start(out[0:128, l0:l0 + RT], sta[:])
                        stb = stg.tile([128, RT], f32, tag="stb", name="stb")
                        nc.scalar.activation(stb[lo:hi, :], p2[lo:hi, :],
                                             Act.Identity,
                                             bias=biasp["P2"][lo:hi, :])
                        nc.sync.dma_start(out[128:192, l0:l0 + RT],
                                          stb[lo:hi, :])

                phase_c("P2")
                phase_c("P0")
                phase_d(0)
                phase_c("P1")
                phase_d(1)

    nc.compile()
    return nc


def _get_nc():
    if "nc" not in _BUILT:
        _BUILT["nc"] = build()
    return _BUILT["nc"]


def kernel(x, Wk, bk, Wg, bg, Wx, bx, Wp, bp, dc):
    nc = _get_nc()
    x = np.asarray(x, dtype=np.float32)
    f32 = lambda a: np.ascontiguousarray(np.asarray(a, dtype=np.float32))
    T32 = lambda a: np.ascontiguousarray(np.asarray(a, dtype=np.float32).T)
    f16T = lambda a: np.ascontiguousarray(
        np.asarray(a, dtype=np.float32).T.astype(np.float16))

    WxT = f16T(Wx)
    WpT = f16T(Wp)
    WkT = f16T(Wk)
    wg2 = np.ascontiguousarray(
        (0.5 * np.asarray(Wg, dtype=np.float32)).T.astype(np.float16))
    dup = lambda wT: np.ascontiguousarray(
        np.concatenate([wT[128:192], wT[128:192]], axis=0))
    colv = lambda v, lo, hi: np.ascontiguousarray(
        np.asarray(v, dtype=np.float32)[lo:hi].reshape(-1, 1))
    dup_col = lambda v: np.ascontiguousarray(
        np.concatenate([colv(v, 128, 192), colv(v, 128, 192)], axis=0))

    shared = {
        "wxT_a": WxT[0:128], "wxT_b": dup(WxT),
        "wpT_a": WpT[0:128], "wpT_b": dup(WpT),
        "wkT_a": WkT[0:128], "wkT_b": dup(WkT),
        "wg2": wg2,
        "bx_a": colv(bx, 0, 128), "bx_b": dup_col(bx),
        "bp_a": colv(bp, 0, 128), "bp_b": dup_col(bp),
        "dc_a": colv(dc, 0, 128), "dc_b": dup_col(dc),
        "bk_bc": np.ascontiguousarray(np.tile(f32(bk).reshape(1, C), (9, 1))),
        "bg_bc": np.ascontiguousarray(np.tile(f32(bg).reshape(1, 9), (128, 1))),
    }
    in_maps = []
    for core in range(NCORES):
        m = dict(shared)
        m["x0"] = np.ascontiguousarray(x[2 * core].reshape(C, L))
        m["x1"] = np.ascontiguousarray(x[2 * core + 1].reshape(C, L))
        in_maps.append(m)

    res = bass_utils.run_bass_kernel_spmd(nc, in_maps,
                                          core_ids=list(range(NCORES)))
    out = np.empty((B, C, H, W), dtype=np.float32)
    for core in range(NCORES):
        out[2 * core] = res.results[core]["out0"].reshape(C, H, W)
        out[2 * core + 1] = res.results[core]["out1"].reshape(C, H, W)
    return out
